# revision 1
# baseline (speedup 1.0000x reference)
"""Trainium2 Bass kernel for nn_AdaptiveCapsule (capsule routing).

Strategy (8 NeuronCores, shard in_caps I=1024 -> IL=128 per core):
  u_hat[b,i,o,d] = sum_e W[i,o,d,e] * x[b,i,e]   (34 GFLOP einsum)
  3 routing iterations over out_caps with tiny (64x512 f32) AllReduces.

Device pipeline per core (v4):
  - W and x are passed as uint16 views of the f32 arrays: odd u16 slots
    hold the round-to-nearest bf16 of each f32, even slots are zeroed
    (full f32-sized byte stream still goes through HBM; the zeroing only
    sanitizes the bytes the bf16 pipeline discards so they can never
    decode as Inf/NaN).
  - DMA X-bar transpose (2-byte path) streams W_i / x_i from HBM
    directly into SBUF transposed, so the contraction axis E lands on
    partitions. Tiles interleave [0,bf16] on [even,odd] partitions; the
    matmul contracts K=128 with 64 live rows (0*0=0 on even rows).
  - TensorE: per in-cap pair, two column-tiled (M=64) bf16 matmuls,
    N=512, accumulating 8 K-chunks into one PSUM (128,512) f32 tile.
  - u_hat kept in SBUF as bf16 (128 x PAIRS x 512): partition = batch
    (64) x 2 (i-parity), free = (i-pair, o*d). Routing multiplies run in
    bf16 (DVE 2x mode); reductions accumulate in f32; bf16 partial sums
    grouped (8 pairs) to bound rounding.
  - 3x AllReduce (64x512 f32 = 128KB) over all 8 cores; partition-half
    fold via SWDGE accumulate-DMA.
"""

import sys

sys.path.insert(0, "/opt/trn_rl_repo")

import numpy as np
import ml_dtypes

from concourse import bacc, bass, tile
from concourse import mybir
from concourse.bass_utils import run_bass_kernel_spmd

NCORES = 8
B, I, O, D, E = 64, 1024, 16, 32, 512
IL = I // NCORES  # 128 in_caps per core
OD = O * D  # 512
E2 = 2 * E  # 1024 u16 columns per f32 row
PAIRS = IL // 2  # 64 i-pairs per core
NCHUNK = 8  # K chunks of 128 interleaved partitions (64 live e each)
GK = 8  # pairs per bf16 partial-sum group in the weighted sum
F32 = mybir.dt.float32
BF16 = mybir.dt.bfloat16
AX = mybir.AxisListType
ALU = mybir.AluOpType
ACTF = mybir.ActivationFunctionType

_CACHE = {}


def _softmax(nc, logits, attn, attn_b, mx, sm):
    # logits: (128, PAIRS, O) f32 -> attn f32, attn_b bf16
    nc.vector.tensor_reduce(mx[:], logits[:], axis=AX.X, op=ALU.max)
    nc.vector.tensor_tensor(
        attn[:], logits[:], mx[:].unsqueeze(-1).broadcast_to((128, PAIRS, O)),
        op=ALU.subtract,
    )
    nc.scalar.activation(attn[:], attn[:], ACTF.Exp)
    nc.vector.tensor_reduce(sm[:], attn[:], axis=AX.X, op=ALU.add)
    nc.vector.reciprocal(sm[:], sm[:])
    nc.vector.tensor_tensor(
        attn[:], attn[:], sm[:].unsqueeze(-1).broadcast_to((128, PAIRS, O)),
        op=ALU.mult,
    )
    nc.vector.tensor_copy(attn_b[:], attn[:])


def _squash(nc, s, v, sq, n2, nrm, den):
    # s,v: (64, OD) f32; squash(s) = norm/(1+norm^2) * s along d
    nc.vector.tensor_tensor(sq[:], s[:], s[:], op=ALU.mult)
    nc.vector.tensor_reduce(
        n2[:], sq[:].rearrange("p (o d) -> p o d", o=O), axis=AX.X, op=ALU.add
    )
    nc.scalar.activation(nrm[:], n2[:], ACTF.Sqrt)
    nc.vector.tensor_scalar_add(den[:], n2[:], 1.0)
    nc.vector.reciprocal(den[:], den[:])
    nc.vector.tensor_tensor(nrm[:], nrm[:], den[:], op=ALU.mult)  # factor
    nc.vector.tensor_tensor(
        v[:].rearrange("p (o d) -> p o d", o=O),
        s[:].rearrange("p (o d) -> p o d", o=O),
        nrm[:].unsqueeze(-1).broadcast_to((64, O, D)),
        op=ALU.mult,
    )


def _build():
    nc = bacc.Bacc(None, target_bir_lowering=False, num_devices=NCORES)
    xu = nc.dram_tensor("xu", [B, IL, E2], BF16, kind="ExternalInput")
    wu = nc.dram_tensor("wu", [PAIRS, NCHUNK, 2 * OD, 128], BF16, kind="ExternalInput")
    fin = nc.dram_tensor("fold", [128, 64], BF16, kind="ExternalInput")
    out = nc.dram_tensor("out", [B, OD], F32, kind="ExternalOutput")
    rg = [list(range(NCORES))]

    with tile.TileContext(nc) as tc:
        with (
            tc.tile_pool(name="persist", bufs=1) as persist,
            tc.tile_pool(name="wt", bufs=4) as wpool,
            tc.tile_pool(name="xt", bufs=3) as xpool,
            tc.tile_pool(name="work", bufs=4) as work,
            tc.tile_pool(name="psum", bufs=4, space="PSUM") as psum,
            tc.tile_pool(name="psacc", bufs=2, space="PSUM") as psacc,
            tc.tile_pool(name="dram", bufs=6, space="DRAM") as dram,
        ):
            uhat = persist.tile([128, PAIRS, OD], BF16)
            foldb = persist.tile([128, 64], BF16)
            nc.sync.dma_start(foldb[:], fin[:])
            acc_ps = psacc.tile([64, OD], F32, tag="accps")

            # ---- Phase 1: u_hat via DMA-transposed bf16 matmuls ----
            rings = [nc.sync, nc.scalar]
            for p in range(PAIRS):
                i0 = 2 * p
                xt = xpool.tile([128, 2 * NCHUNK, B], BF16, tag="xt")
                nc.sync.dma_start(
                    out=xt[:], in_=xu[:, i0 : i0 + 2, :], transpose=True
                )
                ps = psum.tile([128, OD], F32, tag="ps")
                # both in-caps of the pair in one 2MB transpose: out chunk t
                # covers u16 cols [128t,128t+128): t<8 -> i0, t>=8 -> i1
                wt = wpool.tile([128, NCHUNK, 2 * OD], BF16, tag="wt")
                for t in range(NCHUNK):
                    nc.sync.dma_start(
                        out=wt[:, t, :], in_=wu[p, t], transpose=True
                    )
                wtv = wt[:].rearrange("p t (i od) -> p t i od", i=2)
                for c in range(NCHUNK):
                    nc.tensor.matmul(
                        ps[0:64, :], xt[:, c, :], wtv[:, c, 0, :],
                        start=(c == 0), stop=(c == NCHUNK - 1),
                        tile_position=(0, 0),
                    )
                for c in range(NCHUNK):
                    nc.tensor.matmul(
                        ps[64:128, :], xt[:, NCHUNK + c, :], wtv[:, c, 1, :],
                        start=(c == 0), stop=(c == NCHUNK - 1),
                        tile_position=(0, 64),
                    )
                nc.vector.tensor_copy(uhat[:, p, :], ps[:])
                # running sum over i for routing iteration 0 (uniform attn):
                # fold matmul sums both partition halves into PSUM f32
                nc.tensor.matmul(
                    acc_ps[:], foldb[:], uhat[:, p, :],
                    start=(p == 0), stop=(p == PAIRS - 1),
                )

            # ---- Routing ----
            logits = persist.tile([128, PAIRS, O], F32)
            attn = persist.tile([128, PAIRS, O], F32)
            attn_b = persist.tile([128, PAIRS, O], BF16)
            mx = persist.tile([128, PAIRS], F32)
            sm = persist.tile([128, PAIRS], F32)
            s_sb = persist.tile([64, OD], F32)
            v_sb = persist.tile([64, OD], F32)
            v_b = persist.tile([64, OD], BF16)
            v_rep = persist.tile([128, OD], BF16)
            sq = persist.tile([64, OD], F32)
            n2 = persist.tile([64, O], F32)
            nrm = persist.tile([64, O], F32)
            den = persist.tile([64, O], F32)
            red16 = work.tile([128, O], F32)

            for it in range(3):
                if it > 0:
                    # logits += sum_d u_hat * v (v_rep aligned with od layout)
                    for p in range(PAIRS):
                        tmpp = work.tile([128, OD], BF16, tag="tmpp")
                        nc.vector.tensor_tensor(
                            tmpp[:], uhat[:, p, :], v_rep[:], op=ALU.mult
                        )
                        if it == 1:
                            nc.vector.tensor_reduce(
                                logits[:, p, :],
                                tmpp[:].rearrange("p (o d) -> p o d", o=O),
                                axis=AX.X, op=ALU.add,
                            )
                        else:
                            nc.vector.tensor_reduce(
                                red16[:],
                                tmpp[:].rearrange("p (o d) -> p o d", o=O),
                                axis=AX.X, op=ALU.add,
                            )
                            nc.vector.tensor_tensor(
                                logits[:, p, :], logits[:, p, :], red16[:],
                                op=ALU.add,
                            )
                    _softmax(nc, logits, attn, attn_b, mx, sm)
                    # s = sum_i attn * u_hat: DVE mult, PE fold-accumulate
                    acc_ps = psacc.tile([64, OD], F32, tag="accps")
                    for p in range(PAIRS):
                        tmpp = work.tile([128, OD], BF16, tag="tmpp")
                        nc.vector.tensor_tensor(
                            tmpp[:].rearrange("p (o d) -> p o d", o=O),
                            uhat[:, p, :].rearrange("p (o d) -> p o d", o=O),
                            attn_b[:, p, :].unsqueeze(-1)
                            .broadcast_to((128, O, D)),
                            op=ALU.mult,
                        )
                        nc.tensor.matmul(
                            acc_ps[:], foldb[:], tmpp[:],
                            start=(p == 0), stop=(p == PAIRS - 1),
                        )

                # drain accumulated s from PSUM (scaled on iter 0), AllReduce
                nc.scalar.mul(s_sb[:], acc_ps[:], (1.0 / O) if it == 0 else 1.0)
                cin = dram.tile([64, OD], F32, tag="cin")
                cout = dram.tile([64, OD], F32, tag="cout")
                nc.sync.dma_start(cin[:], s_sb[:])
                nc.gpsimd.collective_compute(
                    "AllReduce", ALU.add, replica_groups=rg,
                    ins=[cin[:].opt()], outs=[cout[:].opt()],
                )
                nc.sync.dma_start(s_sb[:], cout[:])

                _squash(nc, s_sb, v_sb, sq, n2, nrm, den)
                if it < 2:
                    nc.vector.tensor_copy(v_b[:], v_sb[:])
                    nc.sync.dma_start(v_rep[0:64, :], v_b[:])
                    nc.sync.dma_start(v_rep[64:128, :], v_b[:])

            nc.sync.dma_start(out[:], v_sb[:])

    nc.compile()
    return nc


def _get_nc():
    if "nc" not in _CACHE:
        _CACHE["nc"] = _build()
    return _CACHE["nc"]


def _to_u16_rtn(a):
    """uint16 view of f32 array `a` with odd slots = round-to-nearest-even
    bf16 of each f32 and even (discarded) slots zeroed."""
    u32 = a.view("<u4")
    hi = ((u32 + 0x7FFF + ((u32 >> 16) & 1)) >> 16).astype(np.uint16)
    outu = np.zeros(a.shape[:-1] + (a.shape[-1] * 2,), dtype=np.uint16)
    outu[..., 1::2] = hi
    return outu.view(ml_dtypes.bfloat16)


def _pack_w(w_shard):
    # (IL, OD, E) f32 -> u16 rtn view (IL, OD, 2E) -> chunk-major
    # (PAIRS, NCHUNK, 2*OD, 128) so the xbar reads stream sequentially
    u = _to_u16_rtn(w_shard)  # (IL, OD, 2E) bf16-viewed
    u = u.reshape(PAIRS, 2, OD, NCHUNK, 128)
    u = np.ascontiguousarray(u.transpose(0, 3, 1, 2, 4))
    return u.reshape(PAIRS, NCHUNK, 2 * OD, 128)


def _prep_inputs(x, W, route_bias):
    x = np.ascontiguousarray(np.asarray(x, dtype=np.float32))
    W = np.asarray(W, dtype=np.float32)
    rb = np.asarray(route_bias, dtype=np.float32)
    if np.any(rb):
        W = W + rb  # reference adds the (1,1,O,1,1) bias onto W
    W0 = np.ascontiguousarray(W.reshape(I, OD, E))
    foldm = np.vstack([np.eye(64), np.eye(64)]).astype(ml_dtypes.bfloat16)
    in_maps = []
    for r in range(NCORES):
        sl = slice(r * IL, (r + 1) * IL)
        in_maps.append(
            {
                "wu": _pack_w(np.ascontiguousarray(W0[sl])),
                "xu": _to_u16_rtn(np.ascontiguousarray(x[:, sl, :])),
                "fold": foldm,
            }
        )
    return in_maps


def kernel(x, W, route_bias, _trace=False, _trace_kwargs=None):
    in_maps = _prep_inputs(x, W, route_bias)
    res = run_bass_kernel_spmd(
        _get_nc(), in_maps, core_ids=list(range(NCORES)),
        trace=_trace, **(_trace_kwargs or {}),
    )
    _CACHE["last_results"] = res
    return np.asarray(res.results[0]["out"], dtype=np.float32).reshape(B, O, D)



# revision 11
# speedup vs baseline: 2.2827x; 2.2827x over previous
"""Trainium2 Bass kernel for nn_AdaptiveCapsule (capsule routing).

Strategy (8 NeuronCores, shard in_caps I=1024 -> IL=128 per core):
  u_hat[b,i,o,d] = sum_e W[i,o,d,e] * x[b,i,e]   (34 GFLOP einsum)
  3 routing iterations over out_caps with tiny (64x512 f32) AllReduces.

v5 design (replaces the v4 DMA-transpose/u16-interleave pipeline):
  - Host packs W and x to REAL bf16 (round-to-nearest via astype) in the
    exact transposed SBUF layout (contraction axis E on partitions), so
    the device uses plain large HWDGE DMAs: 32x 2MiB for W, 32x 256KiB
    for x (on the second HWDGE ring). Halves HBM traffic vs v4 and
    avoids the X-bar transpose path (~261 GB/s, descriptor-heavy).
  - TensorE: per in-cap pair, col-tiled (M=64) bf16 matmuls at
    tile_position (0,0)/(0,64), K=128 fully live, 4 K-chunks into one
    (128,512) f32 PSUM tile. Fold matmul (ones-stack lhsT) accumulates
    sum_i u_hat for routing iteration 0 (uniform attn), delayed one pair
    so PE never stalls on the PSUM->SBUF drain.
  - u_hat kept in SBUF bf16 (128 x 64 x 512): partition = batch(64) x
    i-parity(2), free = (pair, o*d).
  - Routing on DVE with 8-pair blocked single instructions (middle-axis
    stride-0 broadcast of v), exp on ScalarE, softmax without max-sub
    (logits are O(30), safe in f32). Weighted i-sum via col-tiled A/B
    fold matmuls (even pairs -> psum[0:64], odd -> psum[64:128]) plus
    one final f32 fold.
  - 3x AllReduce (64x512 f32 = 128KB) over all 8 cores via internal
    DRAM tiles; result loaded into both partition halves for the next
    logits pass.
"""

import sys

sys.path.insert(0, "/opt/trn_rl_repo")

import numpy as np
import ml_dtypes

from concourse import bacc, bass, tile
from concourse import mybir
from concourse.bass_utils import run_bass_kernel_spmd

NCORES = 8
B, I, O, D, E = 64, 1024, 16, 32, 512
IL = I // NCORES  # 128 in_caps per core
OD = O * D  # 512
PAIRS = IL // 2  # 64 i-pairs per core
KCH = E // 128  # 4 contraction chunks of 128
GP = 2  # pairs per W DMA group (2 MiB per DMA)
NG = PAIRS // GP  # 32 groups
BLK = 8  # pairs per routing DVE block
NBLK = PAIRS // BLK
F32 = mybir.dt.float32
BF16 = mybir.dt.bfloat16
AX = mybir.AxisListType
ALU = mybir.AluOpType
ACTF = mybir.ActivationFunctionType

_CACHE = {}


def _build():
    nc = bacc.Bacc(None, target_bir_lowering=False, num_devices=NCORES)
    wp = nc.dram_tensor("wp", [NG, 128, GP * 2 * KCH * OD], BF16, kind="ExternalInput")
    xp = nc.dram_tensor("xp", [NG, 128, GP * 2 * KCH * B], BF16, kind="ExternalInput")
    fin = nc.dram_tensor("fold", [128, 64], BF16, kind="ExternalInput")
    out = nc.dram_tensor("out", [B, OD], F32, kind="ExternalOutput")
    rg = [list(range(NCORES))]

    with tile.TileContext(nc) as tc:
        with (
            tc.tile_pool(name="persist", bufs=1) as persist,
            tc.tile_pool(name="wt", bufs=4) as wpool,
            tc.tile_pool(name="xt", bufs=3) as xpool,
            tc.tile_pool(name="tmp", bufs=2) as tmpool,
            tc.tile_pool(name="tmpg", bufs=2) as tmpg,
            tc.tile_pool(name="psum", bufs=3, space="PSUM") as psum,
            tc.tile_pool(name="psacc", bufs=2, space="PSUM") as psacc,
            tc.tile_pool(name="psfold", bufs=2, space="PSUM") as psfold,
            tc.tile_pool(name="dram", bufs=6, space="DRAM") as dram,
        ):
            uhat = persist.tile([128, PAIRS, OD], BF16)
            foldb = persist.tile([128, 64], BF16)
            foldf = persist.tile([128, 64], F32)
            nc.sync.dma_start(foldb[:], fin[:])
            nc.vector.tensor_copy(foldf[:], foldb[:])
            acc_ps = psacc.tile([64, OD], F32, tag="accps")

            # ---- Phase 1: u_hat via pre-transposed bf16 matmuls ----
            # W/x DMAs alternate the two HWDGE rings (sync/scalar) so the
            # per-DMA fixed costs overlap; all PSUM drains go to VectorE so
            # the scalar ring's queue never blocks a W DMA behind a drain.
            prev = None
            for g in range(NG):
                ring_w = nc.sync if g % 2 == 0 else nc.scalar
                ring_x = nc.scalar if g % 2 == 0 else nc.sync
                wt = wpool.tile([128, GP * 2, KCH, OD], BF16, tag="wt")
                ring_w.dma_start(wt[:], wp[g])
                xt = xpool.tile([128, GP * 2, KCH, B], BF16, tag="xt")
                ring_x.dma_start(xt[:], xp[g])
                for lp in range(GP):
                    p = g * GP + lp
                    ps = psum.tile([128, OD], F32, tag="ps")
                    for c in range(KCH):
                        nc.tensor.matmul(
                            ps[0:64, :], xt[:, 2 * lp, c, :], wt[:, 2 * lp, c, :],
                            start=(c == 0), stop=(c == KCH - 1),
                            tile_position=(0, 0),
                        )
                        nc.tensor.matmul(
                            ps[64:128, :], xt[:, 2 * lp + 1, c, :],
                            wt[:, 2 * lp + 1, c, :],
                            start=(c == 0), stop=(c == KCH - 1),
                            tile_position=(0, 64),
                        )
                    nc.vector.tensor_copy(uhat[:, p, :], ps[:])
                    # fold (i-sum for iter-0 s) delayed one pair so the PE
                    # never waits on the drain of the pair it just computed
                    if prev is not None:
                        nc.tensor.matmul(
                            acc_ps[:], foldb[:], uhat[:, prev, :],
                            start=(prev == 0), stop=False,
                        )
                    prev = p
            nc.tensor.matmul(
                acc_ps[:], foldb[:], uhat[:, prev, :], start=False, stop=True
            )

            # ---- Routing ----
            # logits live in fp16 (10 mantissa bits ~= 1e-3 rel on |l|<60):
            # keeps the big TENSOR_REDUCEs in DVE 2x mode (all-2B operands).
            FP16 = mybir.dt.float16
            logits = persist.tile([128, PAIRS, O], FP16)
            red = persist.tile([128, PAIRS, O], FP16)
            attn32 = persist.tile([128, PAIRS, O], F32)
            attn_b = persist.tile([128, PAIRS, O], BF16)
            sm = persist.tile([128, PAIRS], F32)
            mx = persist.tile([128, PAIRS], FP16)
            accsb = persist.tile([128, OD], F32)
            sprep = persist.tile([128, OD], F32)
            v_rep = persist.tile([128, OD], BF16)
            v_out = persist.tile([64, OD], F32)
            s_sb = persist.tile([64, OD], F32)
            sq = persist.tile([128, OD], F32)
            n2 = persist.tile([128, O], F32)
            nrm = persist.tile([128, O], F32)
            den = persist.tile([128, O], F32)

            GPBLK_L = (6, 7)  # logits-mult blocks offloaded to GpSimd
            GPBLK_S = (6, 7)  # s-mult blocks offloaded to GpSimd

            def logits_mult(eng, blk, pool, tag):
                sl = slice(blk * BLK, (blk + 1) * BLK)
                tmp = pool.tile([128, BLK, OD], BF16, tag=tag, name=f"tl{blk}")
                eng.tensor_tensor(
                    tmp[:], uhat[:, sl, :],
                    v_rep[:].unsqueeze(1).broadcast_to((128, BLK, OD)),
                    op=ALU.mult,
                )
                return tmp

            def logits_red(blk, tmp, it):
                sl = slice(blk * BLK, (blk + 1) * BLK)
                tgt = logits if it == 1 else red
                with nc.allow_low_precision(reason="fp16 logits, |l|<60"):
                    nc.vector.tensor_reduce(
                        tgt[:, sl, :],
                        tmp[:].rearrange("p a (o d) -> p a o d", o=O),
                        axis=AX.X, op=ALU.add,
                    )

            def s_mult(eng, blk, pool, tag):
                sl = slice(blk * BLK, (blk + 1) * BLK)
                tmp = pool.tile([128, BLK, OD], BF16, tag=tag, name=f"ts{blk}")
                eng.tensor_tensor(
                    tmp[:].rearrange("p a (o d) -> p a o d", o=O),
                    uhat[:, sl, :].rearrange("p a (o d) -> p a o d", o=O),
                    attn_b[:, sl, :].unsqueeze(-1)
                    .broadcast_to((128, BLK, O, D)),
                    op=ALU.mult,
                )
                return tmp

            def s_fold(blk, tmp, acc2):
                for lp in range(BLK):
                    p = blk * BLK + lp
                    half = acc2[0:64, :] if p % 2 == 0 else acc2[64:128, :]
                    nc.tensor.matmul(
                        half, foldb[:], tmp[:, lp, :],
                        start=(p < 2), stop=(p >= PAIRS - 2),
                        tile_position=((0, 0) if p % 2 == 0 else (0, 64)),
                    )

            for it in range(3):
                if it > 0:
                    # logits (+)= sum_d u_hat * v; gp blocks issued first so
                    # their DVE reduces (queued last) never stall the FIFO
                    gtmps = [
                        (b, logits_mult(nc.gpsimd, b, tmpg, "tmpg"))
                        for b in GPBLK_L
                    ]
                    for blk in range(NBLK):
                        if blk in GPBLK_L:
                            continue
                        tmp = logits_mult(nc.vector, blk, tmpool, "tmp")
                        logits_red(blk, tmp, it)
                    for b, tmp in gtmps:
                        logits_red(b, tmp, it)
                    if it == 2:
                        with nc.allow_low_precision(reason="fp16 logits"):
                            nc.vector.tensor_tensor(
                                logits[:], logits[:], red[:], op=ALU.add
                            )
                    # softmax over o (max-sub: tail logits exceed exp's f32
                    # range at ~4.5 sigma, so subtract the per-(b,i) max)
                    nc.vector.tensor_reduce(
                        mx[:], logits[:], axis=AX.X, op=ALU.max
                    )
                    nc.vector.tensor_tensor(
                        red[:], logits[:],
                        mx[:].unsqueeze(-1).broadcast_to((128, PAIRS, O)),
                        op=ALU.subtract,
                    )
                    nc.scalar.activation(attn32[:], red[:], ACTF.Exp)
                    nc.vector.tensor_reduce(
                        sm[:], attn32[:], axis=AX.X, op=ALU.add
                    )
                    nc.vector.reciprocal(sm[:], sm[:])
                    nc.vector.tensor_tensor(
                        attn_b[:], attn32[:],
                        sm[:].unsqueeze(-1).broadcast_to((128, PAIRS, O)),
                        op=ALU.mult,
                    )
                    # s = sum_i attn * u_hat: DVE/GpSimd mult, A/B col folds
                    acc2 = psacc.tile([128, OD], F32, tag="accps")
                    gtmps = [
                        (b, s_mult(nc.gpsimd, b, tmpg, "tmpg"))
                        for b in GPBLK_S
                    ]
                    for blk in range(NBLK):
                        if blk in GPBLK_S:
                            continue
                        tmp = s_mult(nc.vector, blk, tmpool, "tmp")
                        s_fold(blk, tmp, acc2)
                    for b, tmp in gtmps:
                        s_fold(b, tmp, acc2)
                    nc.vector.tensor_copy(accsb[:], acc2[:])
                    s_ps = psfold.tile([64, OD], F32, tag="fold")
                    nc.tensor.matmul(
                        s_ps[:], foldf[:], accsb[:], start=True, stop=True
                    )
                    nc.scalar.mul(s_sb[:], s_ps[:], 1.0)
                else:
                    nc.scalar.mul(s_sb[:], acc_ps[:], 1.0 / O)

                cin = dram.tile([64, OD], F32, tag="cin")
                cout = dram.tile([64, OD], F32, tag="cout")
                nc.sync.dma_start(cin[:], s_sb[:])
                nc.gpsimd.collective_compute(
                    "AllReduce", ALU.add, replica_groups=rg,
                    ins=[cin[:].opt()], outs=[cout[:].opt()],
                )
                nc.sync.dma_start(sprep[0:64, :], cout[:])
                if it < 2:
                    nc.scalar.dma_start(sprep[64:128, :], cout[:])

                # squash(s) = norm/(1+norm^2) * s along d, on both halves
                nc.vector.tensor_tensor(sq[:], sprep[:], sprep[:], op=ALU.mult)
                nc.vector.tensor_reduce(
                    n2[:], sq[:].rearrange("p (o d) -> p o d", o=O),
                    axis=AX.X, op=ALU.add,
                )
                nc.scalar.activation(nrm[:], n2[:], ACTF.Sqrt)
                nc.vector.tensor_scalar_add(den[:], n2[:], 1.0)
                nc.vector.reciprocal(den[:], den[:])
                nc.vector.tensor_tensor(nrm[:], nrm[:], den[:], op=ALU.mult)
                if it < 2:
                    nc.vector.tensor_tensor(
                        v_rep[:].rearrange("p (o d) -> p o d", o=O),
                        sprep[:].rearrange("p (o d) -> p o d", o=O),
                        nrm[:].unsqueeze(-1).broadcast_to((128, O, D)),
                        op=ALU.mult,
                    )
                else:
                    nc.vector.tensor_tensor(
                        v_out[:].rearrange("p (o d) -> p o d", o=O),
                        sprep[0:64, :].rearrange("p (o d) -> p o d", o=O),
                        nrm[0:64, :].unsqueeze(-1).broadcast_to((64, O, D)),
                        op=ALU.mult,
                    )
                    nc.sync.dma_start(out[:], v_out[:])

    nc.compile()
    return nc


def _get_nc():
    if "nc" not in _CACHE:
        _CACHE["nc"] = _build()
    return _CACHE["nc"]


def _pack_w(w_shard):
    # (IL, OD, E) f32 -> bf16 (NG, 128, GP*2*KCH*OD), partition = e%128
    wb = w_shard.astype(ml_dtypes.bfloat16)
    wb = wb.reshape(IL, OD, KCH, 128)
    wb = wb.transpose(0, 3, 2, 1)  # (i, part, c, od)
    wb = wb.reshape(NG, GP * 2, 128, KCH, OD)
    wb = np.ascontiguousarray(wb.transpose(0, 2, 1, 3, 4))
    return wb.reshape(NG, 128, GP * 2 * KCH * OD)


def _pack_x(x_shard):
    # (B, IL, E) f32 -> bf16 (NG, 128, GP*2*KCH*B), partition = e%128
    xb = x_shard.astype(ml_dtypes.bfloat16)
    xb = xb.reshape(B, IL, KCH, 128)
    xb = xb.transpose(1, 3, 2, 0)  # (i, part, c, b)
    xb = xb.reshape(NG, GP * 2, 128, KCH, B)
    xb = np.ascontiguousarray(xb.transpose(0, 2, 1, 3, 4))
    return xb.reshape(NG, 128, GP * 2 * KCH * B)


def _prep_inputs(x, W, route_bias):
    x = np.ascontiguousarray(np.asarray(x, dtype=np.float32))
    W = np.asarray(W, dtype=np.float32)
    rb = np.asarray(route_bias, dtype=np.float32)
    if np.any(rb):
        W = W + rb  # reference adds the (1,1,O,1,1) bias onto W
    W0 = np.ascontiguousarray(W.reshape(I, OD, E))
    foldm = np.vstack([np.eye(64), np.eye(64)]).astype(ml_dtypes.bfloat16)
    in_maps = []
    for r in range(NCORES):
        sl = slice(r * IL, (r + 1) * IL)
        in_maps.append(
            {
                "wp": _pack_w(np.ascontiguousarray(W0[sl])),
                "xp": _pack_x(np.ascontiguousarray(x[:, sl, :])),
                "fold": foldm,
            }
        )
    return in_maps


def kernel(x, W, route_bias, _trace=False, _trace_kwargs=None):
    in_maps = _prep_inputs(x, W, route_bias)
    res = run_bass_kernel_spmd(
        _get_nc(), in_maps, core_ids=list(range(NCORES)),
        trace=_trace, **(_trace_kwargs or {}),
    )
    _CACHE["last_results"] = res
    return np.asarray(res.results[0]["out"], dtype=np.float32).reshape(B, O, D)


# revision 12
# speedup vs baseline: 2.6697x; 1.1695x over previous
"""Trainium2 Bass kernel for nn_AdaptiveCapsule (capsule routing).

Strategy (8 NeuronCores, shard in_caps I=1024 -> IL=128 per core):
  u_hat[b,i,o,d] = sum_e W[i,o,d,e] * x[b,i,e]   (34 GFLOP einsum)
  3 routing iterations over out_caps with tiny (64x512 f32) AllReduces.

v7 design:
  - Host packs W and x to fp16 (10 mantissa bits: ~4x less quantization
    noise than bf16) in the exact transposed SBUF layout (contraction
    axis E on partitions), so the device uses plain large HWDGE DMAs
    alternating both rings (sync/scalar): 32x 2MiB for W + 32x 256KiB
    for x. No X-bar transpose, half the bytes of the u16 pipeline.
  - TensorE: per in-cap pair, col-tiled (M=64) fp16 matmuls at
    tile_position (0,0)/(0,64), K=128 fully live, 4 K-chunks into one
    (128,512) f32 PSUM tile; the two column halves stream concurrently.
    Fold matmul (ones-stack lhsT) accumulates sum_i u_hat for routing
    iteration 0 (uniform attn), delayed one pair so the PE never stalls
    on the PSUM->SBUF drain (all drains on VectorE so the scalar ring
    never queues a drain in front of a W DMA).
  - u_hat kept in SBUF fp16 as (128, pair, D*O) with free axis in
    d-major order (od = d*O + o). That makes BOTH routing multiplies
    DVE 2x-mode eligible: v broadcast is middle-axis (inner step 1) and
    attn broadcast is over d (o contiguous inner). The d-reduction for
    logits runs as in-place fp16 tree-adds over the middle axis (2x),
    final stage emitting f32 logits.
  - softmax: per-(b,i) max-sub (tail logits overflow exp otherwise),
    exp on ScalarE, attn normalized to fp16.
  - s = sum_i attn*u_hat via A/B col-tiled fold matmuls (even pairs ->
    psum[0:64], odd -> psum[64:128]) plus one final f32 fold, then
    AllReduce (64x512 f32) via internal DRAM tiles; the result loads
    into both partition halves and squash runs on all 128 partitions.
  - Output leaves in d-major order; the host transposes back to (B,O,D).
"""

import sys

sys.path.insert(0, "/opt/trn_rl_repo")

import numpy as np

from concourse import bacc, bass, tile
from concourse import mybir
from concourse.bass_utils import run_bass_kernel_spmd

NCORES = 8
B, I, O, D, E = 64, 1024, 16, 32, 512
IL = I // NCORES  # 128 in_caps per core
OD = O * D  # 512
PAIRS = IL // 2  # 64 i-pairs per core
KCH = E // 128  # 4 contraction chunks of 128
GP = 2  # pairs per W DMA group (2 MiB per DMA)
NG = PAIRS // GP  # 32 groups
BLK = 8  # pairs per routing DVE block
NBLK = PAIRS // BLK
F32 = mybir.dt.float32
FP16 = mybir.dt.float16
AX = mybir.AxisListType
ALU = mybir.AluOpType
ACTF = mybir.ActivationFunctionType

_CACHE = {}


def _build():
    nc = bacc.Bacc(None, target_bir_lowering=False, num_devices=NCORES)
    wp = nc.dram_tensor("wp", [NG, 128, GP * 2 * KCH * OD], FP16, kind="ExternalInput")
    xp = nc.dram_tensor("xp", [NG, 128, GP * 2 * KCH * B], FP16, kind="ExternalInput")
    fin = nc.dram_tensor("fold", [128, 64], FP16, kind="ExternalInput")
    out = nc.dram_tensor("out", [B, OD], F32, kind="ExternalOutput")
    rg = [list(range(NCORES))]

    with tile.TileContext(nc) as tc:
        with (
            tc.tile_pool(name="persist", bufs=1) as persist,
            tc.tile_pool(name="wt", bufs=5) as wpool,
            tc.tile_pool(name="xt", bufs=3) as xpool,
            tc.tile_pool(name="tmp", bufs=2) as tmpool,
            tc.tile_pool(name="psum", bufs=3, space="PSUM") as psum,
            tc.tile_pool(name="psacc", bufs=2, space="PSUM") as psacc,
            tc.tile_pool(name="psfold", bufs=2, space="PSUM") as psfold,
            tc.tile_pool(name="dram", bufs=6, space="DRAM") as dram,
        ):
            uhat = persist.tile([128, PAIRS, OD], FP16)
            foldb = persist.tile([128, 64], FP16)
            foldf = persist.tile([128, 64], F32)
            nc.sync.dma_start(foldb[:], fin[:])
            nc.vector.tensor_copy(foldf[:], foldb[:])
            acc_ps = psacc.tile([64, OD], F32, tag="accps")

            # ---- Phase 1: u_hat via pre-transposed fp16 matmuls ----
            prev = None
            for g in range(NG):
                ring_w = nc.sync if g % 2 == 0 else nc.scalar
                ring_x = nc.scalar if g % 2 == 0 else nc.sync
                wt = wpool.tile([128, GP * 2, KCH, OD], FP16, tag="wt")
                ring_w.dma_start(wt[:], wp[g])
                xt = xpool.tile([128, GP * 2, KCH, B], FP16, tag="xt")
                ring_x.dma_start(xt[:], xp[g])
                for lp in range(GP):
                    p = g * GP + lp
                    ps = psum.tile([128, OD], F32, tag="ps")
                    for c in range(KCH):
                        nc.tensor.matmul(
                            ps[0:64, :], xt[:, 2 * lp, c, :], wt[:, 2 * lp, c, :],
                            start=(c == 0), stop=(c == KCH - 1),
                            tile_position=(0, 0),
                        )
                        nc.tensor.matmul(
                            ps[64:128, :], xt[:, 2 * lp + 1, c, :],
                            wt[:, 2 * lp + 1, c, :],
                            start=(c == 0), stop=(c == KCH - 1),
                            tile_position=(0, 64),
                        )
                    nc.vector.tensor_copy(uhat[:, p, :], ps[:])
                    # fold (i-sum for iter-0 s) delayed one pair so the PE
                    # never waits on the drain of the pair it just computed
                    if prev is not None:
                        nc.tensor.matmul(
                            acc_ps[:], foldb[:], uhat[:, prev, :],
                            start=(prev == 0), stop=False,
                        )
                    prev = p
            nc.tensor.matmul(
                acc_ps[:], foldb[:], uhat[:, prev, :], start=False, stop=True
            )

            # ---- Routing ----
            logits = persist.tile([128, PAIRS, O], F32)
            red = persist.tile([128, PAIRS, O], F32)
            attn32 = persist.tile([128, PAIRS, O], F32)
            attn_h = persist.tile([128, PAIRS, O], FP16)
            sm = persist.tile([128, PAIRS], F32)
            mx = persist.tile([128, PAIRS], F32)
            accsb = persist.tile([128, OD], F32)
            sprep = persist.tile([128, OD], F32)
            v_rep = persist.tile([128, OD], FP16)
            v_out = persist.tile([64, OD], F32)
            s_sb = persist.tile([64, OD], F32)
            sq = persist.tile([128, OD], F32)
            nrm = persist.tile([128, O], F32)
            den = persist.tile([128, O], F32)

            def logits_block(blk, it):
                # tmp = u_hat * v (2x), then tree-add over d (2x, in-place),
                # final stage writes f32 logits
                sl = slice(blk * BLK, (blk + 1) * BLK)
                tmp = tmpool.tile([128, BLK, OD], FP16, tag="tmp", name=f"tl{blk}")
                nc.vector.tensor_tensor(
                    tmp[:], uhat[:, sl, :],
                    v_rep[:].unsqueeze(1).broadcast_to((128, BLK, OD)),
                    op=ALU.mult,
                )
                v4 = tmp[:].rearrange("p a (d o) -> p a d o", d=D)
                w = D
                while w > 2:
                    h = w // 2
                    nc.vector.tensor_tensor(
                        v4[:, :, 0:h, :], v4[:, :, 0:h, :], v4[:, :, h:w, :],
                        op=ALU.add,
                    )
                    w = h
                tgt = logits if it == 1 else red
                nc.vector.tensor_tensor(
                    tgt[:, sl, :], v4[:, :, 0, :], v4[:, :, 1, :], op=ALU.add
                )

            def s_block(blk, acc2):
                # tmp = u_hat * attn (2x: o is the contiguous inner axis),
                # then A/B col-tiled fold matmuls accumulate sum_i
                sl = slice(blk * BLK, (blk + 1) * BLK)
                tmp = tmpool.tile([128, BLK, OD], FP16, tag="tmp", name=f"ts{blk}")
                nc.vector.tensor_tensor(
                    tmp[:].rearrange("p a (d o) -> p a d o", d=D),
                    uhat[:, sl, :].rearrange("p a (d o) -> p a d o", d=D),
                    attn_h[:, sl, :].unsqueeze(2).broadcast_to((128, BLK, D, O)),
                    op=ALU.mult,
                )
                for lp in range(BLK):
                    p = blk * BLK + lp
                    half = acc2[0:64, :] if p % 2 == 0 else acc2[64:128, :]
                    nc.tensor.matmul(
                        half, foldb[:], tmp[:, lp, :],
                        start=(p < 2), stop=(p >= PAIRS - 2),
                        tile_position=((0, 0) if p % 2 == 0 else (0, 64)),
                    )

            for it in range(3):
                if it > 0:
                    for blk in range(NBLK):
                        logits_block(blk, it)
                    if it == 2:
                        nc.vector.tensor_tensor(
                            logits[:], logits[:], red[:], op=ALU.add
                        )
                    # softmax over o (max-sub: tail logits exceed exp's f32
                    # range at ~4.5 sigma, so subtract the per-(b,i) max)
                    nc.vector.tensor_reduce(
                        mx[:], logits[:], axis=AX.X, op=ALU.max
                    )
                    nc.vector.tensor_tensor(
                        red[:], logits[:],
                        mx[:].unsqueeze(-1).broadcast_to((128, PAIRS, O)),
                        op=ALU.subtract,
                    )
                    nc.scalar.activation(attn32[:], red[:], ACTF.Exp)
                    nc.vector.tensor_reduce(
                        sm[:], attn32[:], axis=AX.X, op=ALU.add
                    )
                    nc.vector.reciprocal(sm[:], sm[:])
                    nc.vector.tensor_tensor(
                        attn_h[:], attn32[:],
                        sm[:].unsqueeze(-1).broadcast_to((128, PAIRS, O)),
                        op=ALU.mult,
                    )
                    acc2 = psacc.tile([128, OD], F32, tag="accps")
                    for blk in range(NBLK):
                        s_block(blk, acc2)
                    nc.vector.tensor_copy(accsb[:], acc2[:])
                    s_ps = psfold.tile([64, OD], F32, tag="fold")
                    nc.tensor.matmul(
                        s_ps[:], foldf[:], accsb[:], start=True, stop=True
                    )
                    nc.scalar.mul(s_sb[:], s_ps[:], 1.0)
                else:
                    nc.scalar.mul(s_sb[:], acc_ps[:], 1.0 / O)

                cin = dram.tile([64, OD], F32, tag="cin")
                cout = dram.tile([64, OD], F32, tag="cout")
                nc.sync.dma_start(cin[:], s_sb[:])
                nc.gpsimd.collective_compute(
                    "AllReduce", ALU.add, replica_groups=rg,
                    ins=[cin[:].opt()], outs=[cout[:].opt()],
                )
                nc.sync.dma_start(sprep[0:64, :], cout[:])
                if it < 2:
                    nc.scalar.dma_start(sprep[64:128, :], cout[:])

                # squash(s) = norm/(1+norm^2) * s along d (d-major layout:
                # square then tree-add over the middle d axis)
                nc.vector.tensor_tensor(sq[:], sprep[:], sprep[:], op=ALU.mult)
                sqv = sq[:].rearrange("p (d o) -> p d o", d=D)
                w = D
                while w > 1:
                    h = w // 2
                    nc.vector.tensor_tensor(
                        sqv[:, 0:h, :], sqv[:, 0:h, :], sqv[:, h:w, :],
                        op=ALU.add,
                    )
                    w = h
                n2 = sqv[:, 0, :]  # (128, O)
                nc.scalar.activation(nrm[:], n2, ACTF.Sqrt)
                nc.vector.tensor_scalar_add(den[:], n2, 1.0)
                nc.vector.reciprocal(den[:], den[:])
                nc.vector.tensor_tensor(nrm[:], nrm[:], den[:], op=ALU.mult)
                if it < 2:
                    nc.vector.tensor_tensor(
                        v_rep[:].rearrange("p (d o) -> p d o", d=D),
                        sprep[:].rearrange("p (d o) -> p d o", d=D),
                        nrm[:].unsqueeze(1).broadcast_to((128, D, O)),
                        op=ALU.mult,
                    )
                else:
                    nc.vector.tensor_tensor(
                        v_out[:].rearrange("p (d o) -> p d o", d=D),
                        sprep[0:64, :].rearrange("p (d o) -> p d o", d=D),
                        nrm[0:64, :].unsqueeze(1).broadcast_to((64, D, O)),
                        op=ALU.mult,
                    )
                    nc.sync.dma_start(out[:], v_out[:])

    nc.compile()
    return nc


def _get_nc():
    if "nc" not in _CACHE:
        _CACHE["nc"] = _build()
    return _CACHE["nc"]


def _pack_w(w_shard):
    # (IL, OD_do, E) f32 (already d-major) -> fp16 (NG, 128, GP*2*KCH*OD)
    wb = w_shard.astype(np.float16)
    wb = wb.reshape(IL, OD, KCH, 128)
    wb = wb.transpose(0, 3, 2, 1)  # (i, part, c, od)
    wb = wb.reshape(NG, GP * 2, 128, KCH, OD)
    wb = np.ascontiguousarray(wb.transpose(0, 2, 1, 3, 4))
    return wb.reshape(NG, 128, GP * 2 * KCH * OD)


def _pack_x(x_shard):
    # (B, IL, E) f32 -> fp16 (NG, 128, GP*2*KCH*B), partition = e%128
    xb = x_shard.astype(np.float16)
    xb = xb.reshape(B, IL, KCH, 128)
    xb = xb.transpose(1, 3, 2, 0)  # (i, part, c, b)
    xb = xb.reshape(NG, GP * 2, 128, KCH, B)
    xb = np.ascontiguousarray(xb.transpose(0, 2, 1, 3, 4))
    return xb.reshape(NG, 128, GP * 2 * KCH * B)


def _prep_inputs(x, W, route_bias):
    x = np.ascontiguousarray(np.asarray(x, dtype=np.float32))
    W = np.asarray(W, dtype=np.float32)
    rb = np.asarray(route_bias, dtype=np.float32)
    if np.any(rb):
        W = W + rb  # reference adds the (1,1,O,1,1) bias onto W
    # d-major free axis: od = d*O + o
    W0 = np.ascontiguousarray(
        W.reshape(I, O, D, E).transpose(0, 2, 1, 3).reshape(I, OD, E)
    )
    foldm = np.vstack([np.eye(64), np.eye(64)]).astype(np.float16)
    in_maps = []
    for r in range(NCORES):
        sl = slice(r * IL, (r + 1) * IL)
        in_maps.append(
            {
                "wp": _pack_w(np.ascontiguousarray(W0[sl])),
                "xp": _pack_x(np.ascontiguousarray(x[:, sl, :])),
                "fold": foldm,
            }
        )
    return in_maps


def kernel(x, W, route_bias, _trace=False, _trace_kwargs=None):
    in_maps = _prep_inputs(x, W, route_bias)
    res = run_bass_kernel_spmd(
        _get_nc(), in_maps, core_ids=list(range(NCORES)),
        trace=_trace, **(_trace_kwargs or {}),
    )
    _CACHE["last_results"] = res
    # device output is d-major: (B, D, O) -> (B, O, D)
    v = np.asarray(res.results[0]["out"], dtype=np.float32).reshape(B, D, O)
    return np.ascontiguousarray(v.transpose(0, 2, 1))
